# revision 4
# baseline (speedup 1.0000x reference)
"""GQA attention (B=2, T=2048, DIM=2048, NH=32, NKV=8, HD=64) with RoPE, causal,
on 8 TRN2 NeuronCores.

Sharding: data-parallel over B (2) x tensor-parallel over kv-head groups (4).
Core i handles batch i//4 and kv heads {2g, 2g+1} (g = i%4), i.e. q heads
8g..8g+8. wq/wk/wv column-parallel, wo row-parallel; host sums the 4 partial
outputs per batch.

Everything on-device is feature-major ("transposed"): x^T, Q^T, K^T are
[feature, t] so no on-device transposes are needed anywhere:
  QT[d,t] = wq^T x^T;  scoresT[s,q] = (KT slice)^T @ QT;  OT[d,q] = V^T @ PT;
  outT[o,t] = wo^T @ OT.  Host transposes the final [o,t] back to [t,o].

v2 vs baseline:
- Score matmuls for the two kv halves of a slab are emitted adjacently with
  lhsT/rhs at base partitions 0 and 64, so they run concurrently in disjoint
  PE row groups (tile_position (0,0)/(64,0) auto-derived) - one 512-cycle
  span instead of two.
- Causal trimming: the 4 diagonal key blocks of each q chunk only compute
  scores/exp/PV for q >= block offset (N = 512-o); the remaining intra-block
  triangle is zeroed by ONE fused strided mask multiply per (slab, q-chunk)
  on GPSIMD over the diagonal pt tile.
- RoPE: the half-swap is done by partition-shifted DVE muls reading the Q/K
  PSUM directly (legal because only one operand is in SBUF) - no SBUF->SBUF
  swap DMAs and no PSUM->SBUF staging copy.
- 1/denominator: reciprocal_approx_fast reads the PSUM ones-row directly.
- Startup: one batched DMA per tensor (rearranged APs) instead of per-k-tile
  DMAs; per-t-block x loads and outT stores are 1-2 descriptors each.
"""

import numpy as np

B, T, DIM = 2, 2048, 2048
NH, NKV, HD = 32, 8, 64
G = 4            # tensor-parallel groups
QH = NH // G     # 8 local q heads
SLABS = 4
KTILES = DIM // 128
TBLK = T // 512

_CACHE = {}


def _f16(x: np.ndarray) -> np.ndarray:
    return np.ascontiguousarray(x, dtype=np.float32).astype(np.float16)


def _build():
    import concourse.bass as bass
    import concourse.mybir as mybir
    import concourse.tile as tile
    from concourse import bacc

    F32 = mybir.dt.float32
    F16 = mybir.dt.float16
    EXP = mybir.ActivationFunctionType.Exp

    nc = bacc.Bacc("TRN2", target_bir_lowering=False, debug=False, num_devices=8)

    xT = nc.dram_tensor("xT", [DIM, T], F16, kind="ExternalInput").ap()
    wq = nc.dram_tensor("wq", [DIM, QH * HD], F16, kind="ExternalInput").ap()
    wk = nc.dram_tensor("wk", [DIM, 2 * HD], F16, kind="ExternalInput").ap()
    wv = nc.dram_tensor("wv", [DIM, 2 * HD], F16, kind="ExternalInput").ap()
    wo = nc.dram_tensor("wo", [QH * HD, DIM], F16, kind="ExternalInput").ap()
    c4 = nc.dram_tensor("c4", [128, T], F32, kind="ExternalInput").ap()
    s4 = nc.dram_tensor("s4", [128, T], F32, kind="ExternalInput").ap()
    vones = nc.dram_tensor("vones", [128, 16 * 65], F16, kind="ExternalInput").ap()
    msk = nc.dram_tensor("msk", [128, 128], F16, kind="ExternalInput").ap()
    outT = nc.dram_tensor("outT", [DIM, T], F16, kind="ExternalOutput").ap()

    from contextlib import ExitStack

    with tile.TileContext(nc) as tc, ExitStack() as ctx:
        # ---------- persistent tiles ----------
        pers = ctx.enter_context(tc.tile_pool(name="pers", bufs=1))
        KT = pers.tile([128, T], F16, tag="kt", name="kt")
        V0 = pers.tile([128, 16 * 65], F16, tag="v0", name="v0")
        V1 = pers.tile([128, 16 * 65], F16, tag="v1", name="v1")
        MSK = pers.tile([128, 128], F16, tag="msk", name="msk_sb")
        C4 = pers.tile([128, T], F32, tag="c4", name="c4_sb")
        S4 = pers.tile([128, T], F32, tag="s4", name="s4_sb")
        WQ = pers.tile([128, KTILES * 512], F16, tag="wq", name="wq_sb")
        WK = pers.tile([128, KTILES * 128], F16, tag="wk", name="wk_sb")
        WV = pers.tile([128, KTILES * 128], F16, tag="wv", name="wv_sb")
        WO = [pers.tile([128, T], F16, tag=f"wo{s}", name=f"wo{s}") for s in range(SLABS)]

        rot = ctx.enter_context(tc.tile_pool(name="rot", bufs=2))
        work = ctx.enter_context(tc.tile_pool(name="work", bufs=3))
        ptp = ctx.enter_context(tc.tile_pool(name="ptp", bufs=4))
        ptdp = ctx.enter_context(tc.tile_pool(name="ptd", bufs=2))
        misc = ctx.enter_context(tc.tile_pool(name="misc", bufs=2))
        osbp = ctx.enter_context(tc.tile_pool(name="osb", bufs=1))
        xtp = ctx.enter_context(tc.tile_pool(name="xt", bufs=2))
        ps_acc = ctx.enter_context(tc.tile_pool(name="ps_acc", bufs=2, space="PSUM"))
        ps_sc = ctx.enter_context(tc.tile_pool(name="ps_sc", bufs=2, space="PSUM"))
        ps_ot = ctx.enter_context(tc.tile_pool(name="ps_ot", bufs=2, space="PSUM"))

        def load_xts(tb, name):
            t_sl = slice(tb * 512, (tb + 1) * 512)
            xt_t = xtp.tile([128, KTILES * 512], F16, tag="x", name=name)
            src = xT.rearrange("(k p) t -> p k t", p=128)[:, :, t_sl]
            dst = xt_t[:].rearrange("p (k t) -> p k t", k=KTILES)
            nc.sync.dma_start(dst, src)
            return xt_t

        # ---------- startup DMAs: one descriptor per tensor ----------
        xts0 = load_xts(0, "xt0")
        nc.scalar.dma_start(
            WQ[:].rearrange("p (k c) -> p k c", k=KTILES),
            wq.rearrange("(k p) c -> p k c", p=128))
        nc.gpsimd.dma_start(
            WK[:].rearrange("p (k c) -> p k c", k=KTILES),
            wk.rearrange("(k p) c -> p k c", p=128))
        nc.gpsimd.dma_start(
            WV[:].rearrange("p (k c) -> p k c", k=KTILES),
            wv.rearrange("(k p) c -> p k c", p=128))
        nc.gpsimd.dma_start(C4[:], c4)
        nc.gpsimd.dma_start(S4[:], s4)
        nc.gpsimd.dma_start(V0[:], vones)
        nc.gpsimd.dma_start(V1[:], vones)
        nc.gpsimd.dma_start(MSK[:], msk)
        for s in range(SLABS):
            nc.scalar.dma_start(WO[s][:], wo[s * 128:(s + 1) * 128, :])

        def emit_proj(tb, xts):
            """Q/K projections + RoPE + V projection for t block tb."""
            t_sl = slice(tb * 512, (tb + 1) * 512)
            QTr = []
            for s in range(SLABS + 1):
                ps = ps_acc.tile([128, 512], F32, tag="acc", name="pq")
                for k in range(KTILES):
                    if s < SLABS:
                        lhs = WQ[:, k * 512 + s * 128: k * 512 + (s + 1) * 128]
                    else:
                        lhs = WK[:, k * 128:(k + 1) * 128]
                    nc.tensor.matmul(ps[:], lhs, xts[:, k * 512:(k + 1) * 512],
                                     start=(k == 0), stop=(k == KTILES - 1))
                if s < SLABS:
                    dst_t = rot.tile([128, 512], F16, tag=f"qtr{s}", name=f"qtr{s}")
                    QTr.append(dst_t)
                    dst = dst_t[:]
                else:
                    dst = KT[:, t_sl]
                # RoPE: dst = ps*C4 + swap32(ps)*S4 (swap via partition-shifted
                # muls reading the PSUM directly)
                m1 = work.tile([128, 512], F32, tag="m1", name="m1")
                nc.vector.tensor_mul(m1[:], ps[:], C4[:, t_sl])
                m2 = work.tile([128, 512], F32, tag="m2", name="m2")
                for h in (0, 64):
                    nc.vector.tensor_mul(m2[h:h + 32, :], ps[h + 32:h + 64, :],
                                         S4[h:h + 32, t_sl])
                    nc.vector.tensor_mul(m2[h + 32:h + 64, :], ps[h:h + 32, :],
                                         S4[h + 32:h + 64, t_sl])
                nc.vector.tensor_add(dst, m1[:], m2[:])
            for i in range(4):
                sbi = tb * 4 + i
                pv = ps_acc.tile([128, 128], F32, tag="acc", name="pv",
                                 padded_shape=[128, 512])
                for k in range(KTILES):
                    nc.tensor.matmul(pv[:], xts[:, k * 512 + i * 128: k * 512 + (i + 1) * 128],
                                     WV[:, k * 128:(k + 1) * 128],
                                     start=(k == 0), stop=(k == KTILES - 1))
                nc.vector.tensor_copy(V0[:, sbi * 65: sbi * 65 + 64], pv[:, 0:64])
                nc.vector.tensor_copy(V1[:, sbi * 65: sbi * 65 + 64], pv[:, 64:128])
            return QTr

        QTr_by_tb = {0: emit_proj(0, xts0)}
        for tb in range(TBLK):
            t_sl = slice(tb * 512, (tb + 1) * 512)
            if tb + 1 < TBLK:
                xts_n = load_xts(tb + 1, f"xt{tb + 1}")
                QTr_by_tb[tb + 1] = emit_proj(tb + 1, xts_n)
            QTr = QTr_by_tb.pop(tb)

            # ---- attention for q chunk qc = tb ----
            qc = tb
            nblk = (qc + 1) * 4
            OTNr = []
            for s in range(SLABS):
                ot0 = ps_ot.tile([65, 512], F32, tag="ot", name="ot0")
                ot1 = ps_ot.tile([65, 512], F32, tag="ot", name="ot1")

                def score_pair(b, o, sc, ccol):
                    """Packed score matmuls for key block b, q cols [o,512).
                    ccol = column base in the sc tile for half0 (half1 at
                    +512). Emitted adjacently -> concurrent row groups."""
                    nc.tensor.matmul(sc[:, ccol + o:ccol + 512],
                                     KT[0:64, b * 128:(b + 1) * 128],
                                     QTr[s][0:64, o:512], start=True, stop=True)
                    nc.tensor.matmul(sc[:, ccol + 512 + o:ccol + 1024],
                                     KT[64:128, b * 128:(b + 1) * 128],
                                     QTr[s][64:128, o:512], start=True, stop=True)

                def pv_pair(b, o, pt_ap0, pt_ap1, first, last):
                    nc.tensor.matmul(ot0[:, o:512], V0[:, b * 65: b * 65 + 65],
                                     pt_ap0, start=first, stop=last)
                    nc.tensor.matmul(ot1[:, o:512], V1[:, b * 65: b * 65 + 65],
                                     pt_ap1, start=first, stop=last)

                # off-diagonal key blocks: full 512-q pairs
                for b in range(qc * 4):
                    sc = ps_sc.tile([128, 1024], F32, tag="sc", name="sc")
                    score_pair(b, 0, sc[:], 0)
                    pt = ptp.tile([128, 1024], F16, tag="pt", name="pt")
                    nc.scalar.activation(pt[:], sc[:], EXP)
                    pv_pair(b, 0, pt[:, 0:512], pt[:, 512:1024], b == 0, False)

                # diagonal key blocks: trimmed to q >= o, exp into one big
                # tile (block j at col j*1024, tail padding for the strided
                # mask view), then one fused mask multiply per half over the
                # four intra-block triangles at cols j*1152 + h*512 + [0,128)
                ptd = ptdp.tile([128, 5120], F16, tag="ptd", name="ptd")
                for j in range(4):
                    b = qc * 4 + j
                    o = j * 128
                    sc = ps_sc.tile([128, 1024], F32, tag="sc", name="sc")
                    score_pair(b, o, sc[:], 0)
                    nc.scalar.activation(ptd[:, j * 1024 + o: (j + 1) * 1024],
                                         sc[:, o:1024], EXP)
                mb = MSK[:].unsqueeze(1).broadcast_to([128, 4, 128])
                for h in range(2):
                    vh = ptd[:, h * 512: h * 512 + 4 * 1152]
                    vh = vh.rearrange("p (j r) -> p j r", r=1152)[:, :, 0:128]
                    nc.gpsimd.tensor_mul(vh, vh, mb)

                for j in range(4):
                    b = qc * 4 + j
                    o = j * 128
                    c0 = j * 1024
                    pv_pair(b, o, ptd[:, c0 + o: c0 + 512],
                            ptd[:, c0 + 512 + o: c0 + 1024],
                            qc == 0 and j == 0, j == 3)

                # ---- normalize: OTN = ot[0:64] * (1/den) ----
                dsb0 = misc.tile([1, 512], F32, tag="dsb0", name="dsb0")
                dsb1 = misc.tile([1, 512], F32, tag="dsb1", name="dsb1")
                nc.vector.tensor_copy(dsb0[:], ot0[64:65, :])
                nc.vector.tensor_copy(dsb1[:], ot1[64:65, :])
                rcf0 = misc.tile([1, 512], F32, tag="rcf0", name="rcf0")
                rcf1 = misc.tile([1, 512], F32, tag="rcf1", name="rcf1")
                nc.vector.reciprocal_approx_fast(rcf0[:], dsb0[:])
                nc.vector.reciprocal_approx_fast(rcf1[:], dsb1[:])
                # broadcast 1/den: full-128 with rcf1 first, then overwrite
                # rows 0:64 with rcf0 (a 64-row broadcast to base 64 is not
                # reliable on gpsimd)
                bc = misc.tile([128, 512], F32, tag="bc", name="bc")
                nc.gpsimd.partition_broadcast(bc[:], rcf1[:])
                nc.gpsimd.partition_broadcast(bc[0:64, :], rcf0[:])
                otn = rot.tile([128, 512], F16, tag=f"otnr{s}", name=f"otnr{s}")
                OTNr.append(otn)
                nc.vector.tensor_mul(otn[0:64, :], ot0[0:64, :], bc[0:64, :])
                nc.vector.tensor_mul(otn[64:128, :], ot1[0:64, :], bc[64:128, :])

            # ---- output projection for this t chunk ----
            osb = osbp.tile([128, 16 * 512], F16, tag="osb", name="osb")
            for ob in range(16):
                po = ps_acc.tile([128, 512], F32, tag="acc", name="po")
                for s in range(SLABS):
                    nc.tensor.matmul(po[:], WO[s][:, ob * 128:(ob + 1) * 128],
                                     OTNr[s][:], start=(s == 0), stop=(s == SLABS - 1))
                nc.vector.tensor_copy(osb[:, ob * 512:(ob + 1) * 512], po[:])
            outv = outT.rearrange("(o p) t -> p o t", p=128)[:, :, t_sl]
            osbv = osb[:].rearrange("p (o t) -> p o t", o=16)
            nc.sync.dma_start(outv[:, 0:8], osbv[:, 0:8])
            nc.sync.dma_start(outv[:, 8:16], osbv[:, 8:16])

    nc.compile()
    return nc


def _prep_inputs(x, freqs_cos, freqs_sin, wq, wk, wv, wo):
    """Build the 8 per-core input maps (host-side sharding + layout prep)."""
    x = np.asarray(x, dtype=np.float32)
    freqs_cos = np.asarray(freqs_cos, dtype=np.float32)
    freqs_sin = np.asarray(freqs_sin, dtype=np.float32)
    wq = np.asarray(wq, dtype=np.float32)
    wk = np.asarray(wk, dtype=np.float32)
    wv = np.asarray(wv, dtype=np.float32)
    wo = np.asarray(wo, dtype=np.float32)

    # de-interleave permutation within a head: [2j] then [2j+1]
    deint = np.concatenate([np.arange(0, HD, 2), np.arange(1, HD, 2)])

    # rope tables [128, T]: row r uses freq index r % 32; sign of sin flips
    # per 32-block (real-out blocks get -sin)
    cosT = freqs_cos.T  # [32, T]
    sinT = freqs_sin.T
    c4 = np.tile(cosT, (4, 1)).astype(np.float32)
    s4 = np.concatenate([-sinT, sinT, -sinT, sinT], axis=0).astype(np.float32)

    vones = np.zeros((128, 16 * 65), dtype=np.float32)
    vones[:, 64::65] = 1.0
    # intra-block causal triangle: valid (keep) iff key-row p <= q-col c
    msk = (np.arange(128)[:, None] <= np.arange(128)[None, :]).astype(np.float32)

    in_maps = []
    for core in range(8):
        b, g = divmod(core, 4)
        # local q head order: slab-major, (s, half) -> global head 8g + s + 4*half
        qheads = [8 * g + s + 4 * h for s in range(SLABS) for h in range(2)]
        kvheads = [2 * g, 2 * g + 1]

        wq_cols = np.concatenate([qh * HD + deint for qh in qheads])
        wk_cols = np.concatenate([kh * HD + deint for kh in kvheads])
        wv_cols = np.concatenate([np.arange(kh * HD, (kh + 1) * HD) for kh in kvheads])
        wo_rows = np.concatenate([np.arange(qh * HD, (qh + 1) * HD) for qh in qheads])

        in_maps.append({
            "xT": _f16(x[b].T),
            "wq": _f16(wq[:, wq_cols] * (1.0 / np.sqrt(HD))),
            "wk": _f16(wk[:, wk_cols]),
            "wv": _f16(wv[:, wv_cols]),
            "wo": _f16(wo[wo_rows, :]),
            "c4": c4,
            "s4": s4,
            "vones": _f16(vones),
            "msk": _f16(msk),
        })
    return in_maps


def kernel(x, freqs_cos, freqs_sin, wq, wk, wv, wo, _trace=False):
    from concourse.bass_utils import run_bass_kernel_spmd

    if "nc" not in _CACHE:
        _CACHE["nc"] = _build()
    nc = _CACHE["nc"]

    in_maps = _prep_inputs(x, freqs_cos, freqs_sin, wq, wk, wv, wo)
    res = run_bass_kernel_spmd(nc, in_maps, core_ids=list(range(8)), trace=_trace)
    _CACHE["last_result"] = res

    out = np.empty((B, T, DIM), dtype=np.float32)
    for b in range(B):
        acc = res.results[4 * b]["outT"].astype(np.float32)
        for g in range(1, 4):
            acc = acc + res.results[4 * b + g]["outT"].astype(np.float32)
        out[b] = acc.T
    return out


# revision 5
# speedup vs baseline: 1.0345x; 1.0345x over previous
"""GQA attention (B=2, T=2048, DIM=2048, NH=32, NKV=8, HD=64) with RoPE, causal,
on 8 TRN2 NeuronCores.

Sharding: data-parallel over B (2) x tensor-parallel over kv-head groups (4).
Core i handles batch i//4 and kv heads {2g, 2g+1} (g = i%4), i.e. q heads
8g..8g+8. wq/wk/wv column-parallel, wo row-parallel; host sums the 4 partial
outputs per batch.

Everything on-device is feature-major ("transposed"): x^T, Q^T, K^T are
[feature, t] so no on-device transposes are needed anywhere.

v3 structure:
- Score matmuls for the two kv halves of a slab are emitted adjacently with
  operands at base partitions 0 and 64, so they run concurrently in disjoint
  PE row groups (tile_position (0,0)/(64,0)) - one 512-cycle span for both.
- Causal trimming: the diagonal key blocks only compute scores/exp/PV for
  q >= block offset; the intra-block triangle is zeroed by one fused strided
  mask multiply per half per (slab, q-chunk) on GPSIMD.
- RoPE: the half-swap is partition-shifted DVE muls reading the Q/K PSUM
  directly; the final add runs on GPSIMD (SBUF-only operands).
- Attention output is copied out of PSUM (unnormalized, f32) right after the
  last PV so the two PSUM accumulators recycle after ~2 DVE copies instead of
  after the whole normalize chain; normalization then runs on SBUF tiles.
- The projection matmuls for t-block tb+1 and the output projection for tb-1
  are emitted as filler chunks BETWEEN attention blocks of tb, so the PE
  always has independent work while the exp/mask chain of the current block
  completes (the Tile scheduler issues greedily in emission order).
- Startup DMAs are chunked (4 k-tiles each) so the first projection matmuls
  start as soon as the first chunks land.
"""

import numpy as np

B, T, DIM = 2, 2048, 2048
NH, NKV, HD = 32, 8, 64
G = 4            # tensor-parallel groups
QH = NH // G     # 8 local q heads
SLABS = 4
KTILES = DIM // 128
TBLK = T // 512

_CACHE = {}


def _f16(x: np.ndarray) -> np.ndarray:
    return np.ascontiguousarray(x, dtype=np.float32).astype(np.float16)


def _build():
    import concourse.bass as bass
    import concourse.mybir as mybir
    import concourse.tile as tile
    from concourse import bacc

    F32 = mybir.dt.float32
    F16 = mybir.dt.float16
    EXP = mybir.ActivationFunctionType.Exp

    nc = bacc.Bacc("TRN2", target_bir_lowering=False, debug=False, num_devices=8)

    xT = nc.dram_tensor("xT", [DIM, T], F16, kind="ExternalInput").ap()
    wq = nc.dram_tensor("wq", [DIM, QH * HD], F16, kind="ExternalInput").ap()
    wk = nc.dram_tensor("wk", [DIM, 2 * HD], F16, kind="ExternalInput").ap()
    wv = nc.dram_tensor("wv", [DIM, 2 * HD], F16, kind="ExternalInput").ap()
    wo = nc.dram_tensor("wo", [QH * HD, DIM], F16, kind="ExternalInput").ap()
    c4 = nc.dram_tensor("c4", [128, T], F32, kind="ExternalInput").ap()
    s4 = nc.dram_tensor("s4", [128, T], F32, kind="ExternalInput").ap()
    vones = nc.dram_tensor("vones", [128, 16 * 65], F16, kind="ExternalInput").ap()
    msk = nc.dram_tensor("msk", [128, 128], F16, kind="ExternalInput").ap()
    outT = nc.dram_tensor("outT", [DIM, T], F16, kind="ExternalOutput").ap()

    from contextlib import ExitStack

    with tile.TileContext(nc) as tc, ExitStack() as ctx:
        # ---------- persistent tiles ----------
        pers = ctx.enter_context(tc.tile_pool(name="pers", bufs=1))
        KT = pers.tile([128, T], F16, tag="kt", name="kt")
        V0 = pers.tile([128, 16 * 65], F16, tag="v0", name="v0")
        V1 = pers.tile([128, 16 * 65], F16, tag="v1", name="v1")
        MSK = pers.tile([128, 128], F16, tag="msk", name="msk_sb")
        C4 = pers.tile([128, T], F32, tag="c4", name="c4_sb")
        S4 = pers.tile([128, T], F32, tag="s4", name="s4_sb")
        WQ = pers.tile([128, KTILES * 512], F16, tag="wq", name="wq_sb")
        WK = pers.tile([128, KTILES * 128], F16, tag="wk", name="wk_sb")
        WV = pers.tile([128, KTILES * 128], F16, tag="wv", name="wv_sb")
        WO = [pers.tile([128, T], F16, tag=f"wo{s}", name=f"wo{s}") for s in range(SLABS)]

        rot = ctx.enter_context(tc.tile_pool(name="rot", bufs=2))
        work = ctx.enter_context(tc.tile_pool(name="work", bufs=3))
        ptp = ctx.enter_context(tc.tile_pool(name="ptp", bufs=4))
        ptdp = ctx.enter_context(tc.tile_pool(name="ptd", bufs=2))
        misc = ctx.enter_context(tc.tile_pool(name="misc", bufs=2))
        osbp = ctx.enter_context(tc.tile_pool(name="osb", bufs=1))
        xtp = ctx.enter_context(tc.tile_pool(name="xt", bufs=2))
        ps_acc = ctx.enter_context(tc.tile_pool(name="ps_acc", bufs=2, space="PSUM"))
        ps_sc = ctx.enter_context(tc.tile_pool(name="ps_sc", bufs=2, space="PSUM"))
        ps_ot = ctx.enter_context(tc.tile_pool(name="ps_ot", bufs=2, space="PSUM"))

        def load_xts(tb, name, queue, nchunk=1):
            t_sl = slice(tb * 512, (tb + 1) * 512)
            xt_t = xtp.tile([128, KTILES * 512], F16, tag="x", name=name)
            src = xT.rearrange("(k p) t -> p k t", p=128)[:, :, t_sl]
            dst = xt_t[:].rearrange("p (k t) -> p k t", k=KTILES)
            kc = KTILES // nchunk
            for c in range(nchunk):
                queue.dma_start(dst[:, c * kc:(c + 1) * kc], src[:, c * kc:(c + 1) * kc])
            return xt_t

        # ---------- startup DMAs (chunked so first matmuls start early) ----
        xts0 = load_xts(0, "xt0", nc.sync, nchunk=4)
        wqv_src = wq.rearrange("(k p) c -> p k c", p=128)
        wqv_dst = WQ[:].rearrange("p (k c) -> p k c", k=KTILES)
        for c in range(4):
            nc.scalar.dma_start(wqv_dst[:, c * 4:(c + 1) * 4], wqv_src[:, c * 4:(c + 1) * 4])
        nc.gpsimd.dma_start(
            WK[:].rearrange("p (k c) -> p k c", k=KTILES),
            wk.rearrange("(k p) c -> p k c", p=128))
        nc.gpsimd.dma_start(
            WV[:].rearrange("p (k c) -> p k c", k=KTILES),
            wv.rearrange("(k p) c -> p k c", p=128))
        nc.gpsimd.dma_start(C4[:], c4)
        nc.gpsimd.dma_start(S4[:], s4)
        nc.gpsimd.dma_start(V0[:], vones)
        nc.gpsimd.dma_start(V1[:], vones)
        nc.gpsimd.dma_start(MSK[:], msk)
        for s in range(SLABS):
            nc.scalar.dma_start(WO[s][:], wo[s * 128:(s + 1) * 128, :])

        def proj_chunks(tb, xts, out):
            """Generate filler chunks (closures) for t-block tb's projections.
            out: dict to receive {"QTr": [...]} once emitted."""
            t_sl = slice(tb * 512, (tb + 1) * 512)
            out["QTr"] = []

            def group(s):
                ps_box = {}

                def mk_mm(k0):
                    def f():
                        if k0 == 0:
                            ps_box["ps"] = ps_acc.tile([128, 512], F32, tag="acc",
                                                       name=f"pq{tb}_{s}")
                        ps = ps_box["ps"]
                        for k in range(k0, k0 + 4):
                            if s < SLABS:
                                lhs = WQ[:, k * 512 + s * 128: k * 512 + (s + 1) * 128]
                            else:
                                lhs = WK[:, k * 128:(k + 1) * 128]
                            nc.tensor.matmul(ps[:], lhs, xts[:, k * 512:(k + 1) * 512],
                                             start=(k == 0), stop=(k == KTILES - 1))
                    return f

                def rope():
                    ps = ps_box["ps"]
                    if s < SLABS:
                        dst_t = rot.tile([128, 512], F16, tag=f"qtr{s}", name=f"qtr{tb}_{s}")
                        out["QTr"].append(dst_t)
                        dst = dst_t[:]
                    else:
                        dst = KT[:, t_sl]
                    m1 = work.tile([128, 512], F32, tag="m1", name="m1")
                    nc.vector.tensor_mul(m1[:], ps[:], C4[:, t_sl])
                    m2 = work.tile([128, 512], F32, tag="m2", name="m2")
                    for h in (0, 64):
                        nc.vector.tensor_mul(m2[h:h + 32, :], ps[h + 32:h + 64, :],
                                             S4[h:h + 32, t_sl])
                        nc.vector.tensor_mul(m2[h + 32:h + 64, :], ps[h:h + 32, :],
                                             S4[h + 32:h + 64, t_sl])
                    nc.gpsimd.tensor_add(dst, m1[:], m2[:])

                return [mk_mm(0), mk_mm(4), mk_mm(8), mk_mm(12), rope]

            def vgroup(i):
                pv_box = {}

                def mk_mm(k0):
                    def f():
                        if k0 == 0:
                            pv_box["pv"] = ps_acc.tile([128, 128], F32, tag="acc",
                                                       name=f"pv{tb}_{i}",
                                                       padded_shape=[128, 512])
                        pv = pv_box["pv"]
                        for k in range(k0, k0 + 4):
                            nc.tensor.matmul(
                                pv[:], xts[:, k * 512 + i * 128: k * 512 + (i + 1) * 128],
                                WV[:, k * 128:(k + 1) * 128],
                                start=(k == 0), stop=(k == KTILES - 1))
                    return f

                def vcopy():
                    pv = pv_box["pv"]
                    sbi = tb * 4 + i
                    nc.vector.tensor_copy(V0[:, sbi * 65: sbi * 65 + 64], pv[:, 0:64])
                    nc.vector.tensor_copy(V1[:, sbi * 65: sbi * 65 + 64], pv[:, 64:128])
                return [mk_mm(0), mk_mm(4), mk_mm(8), mk_mm(12), vcopy]

            chunks = []
            for s in range(SLABS + 1):
                chunks.extend(group(s))
            for i in range(4):
                chunks.extend(vgroup(i))
            return chunks

        def oproj_chunks(tb, OTNr, t_sl):
            """Filler chunks for the output projection of t-block tb."""
            osb_box = {}

            def mk_ob(ob):
                def f():
                    if ob == 0:
                        osb_box["osb"] = osbp.tile([128, 16 * 512], F16, tag="osb",
                                                   name=f"osb{tb}")
                    po = ps_acc.tile([128, 512], F32, tag="acc", name=f"po{tb}_{ob}")
                    for s in range(SLABS):
                        nc.tensor.matmul(po[:], WO[s][:, ob * 128:(ob + 1) * 128],
                                         OTNr[s][:], start=(s == 0), stop=(s == SLABS - 1))
                    nc.vector.tensor_copy(osb_box["osb"][:, ob * 512:(ob + 1) * 512], po[:])
                    if ob == 7 or ob == 15:
                        h0 = ob - 7
                        outv = outT.rearrange("(o p) t -> p o t", p=128)[:, :, t_sl]
                        osbv = osb_box["osb"][:].rearrange("p (o t) -> p o t", o=16)
                        nc.sync.dma_start(outv[:, h0:ob + 1], osbv[:, h0:ob + 1])
                return f
            return [mk_ob(ob) for ob in range(16)]

        def emit_attention(qc, QTr, filler):
            """Attention for q chunk qc; emits filler chunks between blocks."""
            nblk = (qc + 1) * 4
            total_blocks = SLABS * nblk
            fi = [0]

            def fill(frac):
                want = int(round(frac * len(filler)))
                while fi[0] < want:
                    filler[fi[0]]()
                    fi[0] += 1

            OTNr = []
            bi = 0
            for s in range(SLABS):
                ot0 = ps_ot.tile([65, 512], F32, tag="ot", name=f"ot0_{qc}_{s}")
                ot1 = ps_ot.tile([65, 512], F32, tag="ot", name=f"ot1_{qc}_{s}")

                def score_pair(b, o, sc):
                    nc.tensor.matmul(sc[:, o:512],
                                     KT[0:64, b * 128:(b + 1) * 128],
                                     QTr[s][0:64, o:512], start=True, stop=True)
                    nc.tensor.matmul(sc[:, 512 + o:1024],
                                     KT[64:128, b * 128:(b + 1) * 128],
                                     QTr[s][64:128, o:512], start=True, stop=True)

                def pv_pair(b, o, pt_ap0, pt_ap1, first, last):
                    nc.tensor.matmul(ot0[:, o:512], V0[:, b * 65: b * 65 + 65],
                                     pt_ap0, start=first, stop=last)
                    nc.tensor.matmul(ot1[:, o:512], V1[:, b * 65: b * 65 + 65],
                                     pt_ap1, start=first, stop=last)

                # off-diagonal key blocks
                for b in range(qc * 4):
                    fill(bi / total_blocks); bi += 1
                    sc = ps_sc.tile([128, 1024], F32, tag="sc", name="sc")
                    score_pair(b, 0, sc[:])
                    pt = ptp.tile([128, 1024], F16, tag="pt", name="pt")
                    nc.scalar.activation(pt[:], sc[:], EXP)
                    pv_pair(b, 0, pt[:, 0:512], pt[:, 512:1024], b == 0, False)

                # diagonal key blocks (trimmed) + fused triangle mask
                ptd = ptdp.tile([128, 5120], F16, tag="ptd", name="ptd")
                for j in range(4):
                    fill(bi / total_blocks); bi += 1
                    b = qc * 4 + j
                    o = j * 128
                    sc = ps_sc.tile([128, 1024], F32, tag="sc", name="sc")
                    score_pair(b, o, sc[:])
                    nc.scalar.activation(ptd[:, j * 1024 + o: (j + 1) * 1024],
                                         sc[:, o:1024], EXP)
                mb = MSK[:].unsqueeze(1).broadcast_to([128, 4, 128])
                for h in range(2):
                    vh = ptd[:, h * 512: h * 512 + 4 * 1152]
                    vh = vh.rearrange("p (j r) -> p j r", r=1152)[:, :, 0:128]
                    nc.gpsimd.tensor_mul(vh, vh, mb)
                for j in range(4):
                    b = qc * 4 + j
                    o = j * 128
                    c0 = j * 1024
                    pv_pair(b, o, ptd[:, c0 + o: c0 + 512],
                            ptd[:, c0 + 512 + o: c0 + 1024],
                            qc == 0 and j == 0, j == 3)

                # release ot0/ot1 quickly: copy unnormalized O and den to SBUF
                otu = misc.tile([128, 512], F32, tag="otu", name="otu")
                nc.vector.tensor_copy(otu[0:64, :], ot0[0:64, :])
                nc.vector.tensor_copy(otu[64:128, :], ot1[0:64, :])
                dsb0 = misc.tile([1, 512], F32, tag="dsb0", name="dsb0")
                dsb1 = misc.tile([1, 512], F32, tag="dsb1", name="dsb1")
                nc.vector.tensor_copy(dsb0[:], ot0[64:65, :])
                nc.vector.tensor_copy(dsb1[:], ot1[64:65, :])
                # normalize on SBUF
                rcf0 = misc.tile([1, 512], F32, tag="rcf0", name="rcf0")
                rcf1 = misc.tile([1, 512], F32, tag="rcf1", name="rcf1")
                nc.vector.reciprocal_approx_fast(rcf0[:], dsb0[:])
                nc.vector.reciprocal_approx_fast(rcf1[:], dsb1[:])
                bc = misc.tile([128, 512], F32, tag="bc", name="bc")
                nc.gpsimd.partition_broadcast(bc[:], rcf1[:])
                nc.gpsimd.partition_broadcast(bc[0:64, :], rcf0[:])
                otn = rot.tile([128, 512], F16, tag=f"otnr{s}", name=f"otnr{qc}_{s}")
                OTNr.append(otn)
                nc.gpsimd.tensor_mul(otn[0:64, :], otu[0:64, :], bc[0:64, :])
                nc.gpsimd.tensor_mul(otn[64:128, :], otu[64:128, :], bc[64:128, :])
            fill(1.0)
            return OTNr

        # ---------- main emission ----------
        proj_out = {0: {}}
        chunks0 = proj_chunks(0, xts0, proj_out[0])
        for c in chunks0:
            c()
        OTNr_prev = None
        for tb in range(TBLK):
            t_sl = slice(tb * 512, (tb + 1) * 512)
            filler = []
            if OTNr_prev is not None:
                t_sl_prev = slice((tb - 1) * 512, tb * 512)
                filler.extend(oproj_chunks(tb - 1, OTNr_prev, t_sl_prev))
            if tb + 1 < TBLK:
                xts_n = load_xts(tb + 1, f"xt{tb + 1}", nc.sync)
                proj_out[tb + 1] = {}
                filler.extend(proj_chunks(tb + 1, xts_n, proj_out[tb + 1]))
            QTr = proj_out.pop(tb)["QTr"]
            OTNr_prev = emit_attention(tb, QTr, filler)
        # final output projection (tail)
        for c in oproj_chunks(TBLK - 1, OTNr_prev,
                              slice((TBLK - 1) * 512, TBLK * 512)):
            c()

    nc.compile()
    return nc


def _prep_inputs(x, freqs_cos, freqs_sin, wq, wk, wv, wo):
    """Build the 8 per-core input maps (host-side sharding + layout prep)."""
    x = np.asarray(x, dtype=np.float32)
    freqs_cos = np.asarray(freqs_cos, dtype=np.float32)
    freqs_sin = np.asarray(freqs_sin, dtype=np.float32)
    wq = np.asarray(wq, dtype=np.float32)
    wk = np.asarray(wk, dtype=np.float32)
    wv = np.asarray(wv, dtype=np.float32)
    wo = np.asarray(wo, dtype=np.float32)

    # de-interleave permutation within a head: [2j] then [2j+1]
    deint = np.concatenate([np.arange(0, HD, 2), np.arange(1, HD, 2)])

    cosT = freqs_cos.T  # [32, T]
    sinT = freqs_sin.T
    c4 = np.tile(cosT, (4, 1)).astype(np.float32)
    s4 = np.concatenate([-sinT, sinT, -sinT, sinT], axis=0).astype(np.float32)

    vones = np.zeros((128, 16 * 65), dtype=np.float32)
    vones[:, 64::65] = 1.0
    # intra-block causal triangle: keep iff key-row p <= q-col c
    msk = (np.arange(128)[:, None] <= np.arange(128)[None, :]).astype(np.float32)

    in_maps = []
    for core in range(8):
        b, g = divmod(core, 4)
        qheads = [8 * g + s + 4 * h for s in range(SLABS) for h in range(2)]
        kvheads = [2 * g, 2 * g + 1]

        wq_cols = np.concatenate([qh * HD + deint for qh in qheads])
        wk_cols = np.concatenate([kh * HD + deint for kh in kvheads])
        wv_cols = np.concatenate([np.arange(kh * HD, (kh + 1) * HD) for kh in kvheads])
        wo_rows = np.concatenate([np.arange(qh * HD, (qh + 1) * HD) for qh in qheads])

        in_maps.append({
            "xT": _f16(x[b].T),
            "wq": _f16(wq[:, wq_cols] * (1.0 / np.sqrt(HD))),
            "wk": _f16(wk[:, wk_cols]),
            "wv": _f16(wv[:, wv_cols]),
            "wo": _f16(wo[wo_rows, :]),
            "c4": c4,
            "s4": s4,
            "vones": _f16(vones),
            "msk": _f16(msk),
        })
    return in_maps


def kernel(x, freqs_cos, freqs_sin, wq, wk, wv, wo, _trace=False):
    from concourse.bass_utils import run_bass_kernel_spmd

    if "nc" not in _CACHE:
        _CACHE["nc"] = _build()
    nc = _CACHE["nc"]

    in_maps = _prep_inputs(x, freqs_cos, freqs_sin, wq, wk, wv, wo)
    res = run_bass_kernel_spmd(nc, in_maps, core_ids=list(range(8)), trace=_trace)
    _CACHE["last_result"] = res

    out = np.empty((B, T, DIM), dtype=np.float32)
    for b in range(B):
        acc = res.results[4 * b]["outT"].astype(np.float32)
        for g in range(1, 4):
            acc = acc + res.results[4 * b + g]["outT"].astype(np.float32)
        out[b] = acc.T
    return out


# revision 6
# speedup vs baseline: 1.1400x; 1.1020x over previous
"""GQA attention (B=2, T=2048, DIM=2048, NH=32, NKV=8, HD=64) with RoPE, causal,
on 8 TRN2 NeuronCores.

Sharding: data-parallel over B (2) x tensor-parallel over kv-head groups (4).
Core i handles batch i//4 and kv heads {2g, 2g+1} (g = i%4), i.e. q heads
8g..8g+8. wq/wk/wv column-parallel, wo row-parallel; host sums the 4 partial
outputs per batch.

Everything on-device is feature-major ("transposed"): x^T, Q^T, K^T are
[feature, t] so no on-device transposes are needed anywhere.

v3 structure:
- Score matmuls for the two kv halves of a slab are emitted adjacently with
  operands at base partitions 0 and 64, so they run concurrently in disjoint
  PE row groups (tile_position (0,0)/(64,0)) - one 512-cycle span for both.
- Causal trimming: the diagonal key blocks only compute scores/exp/PV for
  q >= block offset; the intra-block triangle is zeroed by one fused strided
  mask multiply per half per (slab, q-chunk) on GPSIMD.
- RoPE: the half-swap is partition-shifted DVE muls reading the Q/K PSUM
  directly; the final add runs on GPSIMD (SBUF-only operands).
- Attention output is copied out of PSUM (unnormalized, f32) right after the
  last PV so the two PSUM accumulators recycle after ~2 DVE copies instead of
  after the whole normalize chain; normalization then runs on SBUF tiles.
- The projection matmuls for t-block tb+1 and the output projection for tb-1
  are emitted as filler chunks BETWEEN attention blocks of tb, so the PE
  always has independent work while the exp/mask chain of the current block
  completes (the Tile scheduler issues greedily in emission order).
- Startup DMAs are chunked (4 k-tiles each) so the first projection matmuls
  start as soon as the first chunks land.
"""

import numpy as np

B, T, DIM = 2, 2048, 2048
NH, NKV, HD = 32, 8, 64
G = 4            # tensor-parallel groups
QH = NH // G     # 8 local q heads
SLABS = 4
KTILES = DIM // 128
TBLK = T // 512

_CACHE = {}


def _f16(x: np.ndarray) -> np.ndarray:
    return np.ascontiguousarray(x, dtype=np.float32).astype(np.float16)


def _build():
    import concourse.bass as bass
    import concourse.mybir as mybir
    import concourse.tile as tile
    from concourse import bacc

    F32 = mybir.dt.float32
    F16 = mybir.dt.float16
    EXP = mybir.ActivationFunctionType.Exp

    nc = bacc.Bacc("TRN2", target_bir_lowering=False, debug=False, num_devices=8)

    xT = nc.dram_tensor("xT", [DIM, T], F16, kind="ExternalInput").ap()
    wq = nc.dram_tensor("wq", [DIM, QH * HD], F16, kind="ExternalInput").ap()
    wk = nc.dram_tensor("wk", [DIM, 2 * HD], F16, kind="ExternalInput").ap()
    wv = nc.dram_tensor("wv", [DIM, 2 * HD], F16, kind="ExternalInput").ap()
    wo = nc.dram_tensor("wo", [QH * HD, DIM], F16, kind="ExternalInput").ap()
    c4 = nc.dram_tensor("c4", [128, T], F32, kind="ExternalInput").ap()
    s4 = nc.dram_tensor("s4", [128, T], F32, kind="ExternalInput").ap()
    vones = nc.dram_tensor("vones", [128, 16 * 65], F16, kind="ExternalInput").ap()
    msk = nc.dram_tensor("msk", [128, 128], F16, kind="ExternalInput").ap()
    outT = nc.dram_tensor("outT", [DIM, T], F16, kind="ExternalOutput").ap()

    from contextlib import ExitStack

    with tile.TileContext(nc) as tc, ExitStack() as ctx:
        # ---------- persistent tiles ----------
        pers = ctx.enter_context(tc.tile_pool(name="pers", bufs=1))
        KT = pers.tile([128, T], F16, tag="kt", name="kt")
        V0 = pers.tile([128, 16 * 65], F16, tag="v0", name="v0")
        V1 = pers.tile([128, 16 * 65], F16, tag="v1", name="v1")
        MSK = pers.tile([128, 128], F16, tag="msk", name="msk_sb")
        C4 = pers.tile([128, T], F32, tag="c4", name="c4_sb")
        S4 = pers.tile([128, T], F32, tag="s4", name="s4_sb")
        WQ = pers.tile([128, KTILES * 512], F16, tag="wq", name="wq_sb")
        WK = pers.tile([128, KTILES * 128], F16, tag="wk", name="wk_sb")
        WV = pers.tile([128, KTILES * 128], F16, tag="wv", name="wv_sb")
        WO = [pers.tile([128, T], F16, tag=f"wo{s}", name=f"wo{s}") for s in range(SLABS)]

        rot = ctx.enter_context(tc.tile_pool(name="rot", bufs=2))
        work = ctx.enter_context(tc.tile_pool(name="work", bufs=3))
        ptp = ctx.enter_context(tc.tile_pool(name="ptp", bufs=4))
        ptdp = ctx.enter_context(tc.tile_pool(name="ptd", bufs=2))
        misc = ctx.enter_context(tc.tile_pool(name="misc", bufs=2))
        osbp = ctx.enter_context(tc.tile_pool(name="osb", bufs=1))
        xtp = ctx.enter_context(tc.tile_pool(name="xt", bufs=2))
        ps_acc = ctx.enter_context(tc.tile_pool(name="ps_acc", bufs=2, space="PSUM"))
        ps_sc = ctx.enter_context(tc.tile_pool(name="ps_sc", bufs=2, space="PSUM"))
        ps_ot = ctx.enter_context(tc.tile_pool(name="ps_ot", bufs=2, space="PSUM"))

        def load_xts(tb, name, queue, nchunk=1):
            t_sl = slice(tb * 512, (tb + 1) * 512)
            xt_t = xtp.tile([128, KTILES * 512], F16, tag="x", name=name)
            src = xT.rearrange("(k p) t -> p k t", p=128)[:, :, t_sl]
            dst = xt_t[:].rearrange("p (k t) -> p k t", k=KTILES)
            kc = KTILES // nchunk
            for c in range(nchunk):
                queue.dma_start(dst[:, c * kc:(c + 1) * kc], src[:, c * kc:(c + 1) * kc])
            return xt_t

        # ---------- startup DMAs (chunked so first matmuls start early) ----
        xts0 = load_xts(0, "xt0", nc.sync, nchunk=4)
        wqv_src = wq.rearrange("(k p) c -> p k c", p=128)
        wqv_dst = WQ[:].rearrange("p (k c) -> p k c", k=KTILES)
        for c in range(4):
            nc.scalar.dma_start(wqv_dst[:, c * 4:(c + 1) * 4], wqv_src[:, c * 4:(c + 1) * 4])
        nc.gpsimd.dma_start(
            WK[:].rearrange("p (k c) -> p k c", k=KTILES),
            wk.rearrange("(k p) c -> p k c", p=128))
        nc.gpsimd.dma_start(
            WV[:].rearrange("p (k c) -> p k c", k=KTILES),
            wv.rearrange("(k p) c -> p k c", p=128))
        nc.gpsimd.dma_start(C4[:], c4)
        nc.gpsimd.dma_start(S4[:], s4)
        nc.gpsimd.dma_start(V0[:], vones)
        nc.gpsimd.dma_start(V1[:], vones)
        nc.gpsimd.dma_start(MSK[:], msk)
        for s in range(SLABS):
            nc.scalar.dma_start(WO[s][:], wo[s * 128:(s + 1) * 128, :])

        def proj_chunks(tb, xts, out):
            """Generate filler chunks (closures) for t-block tb's projections.
            out: dict to receive {"QTr": [...]} once emitted."""
            t_sl = slice(tb * 512, (tb + 1) * 512)
            out["QTr"] = []

            def group(s):
                ps_box = {}

                def mk_mm(k0):
                    def f():
                        if k0 == 0:
                            ps_box["ps"] = ps_acc.tile([128, 512], F32, tag="acc",
                                                       name=f"pq{tb}_{s}")
                        ps = ps_box["ps"]
                        for k in range(k0, k0 + 4):
                            if s < SLABS:
                                lhs = WQ[:, k * 512 + s * 128: k * 512 + (s + 1) * 128]
                            else:
                                lhs = WK[:, k * 128:(k + 1) * 128]
                            nc.tensor.matmul(ps[:], lhs, xts[:, k * 512:(k + 1) * 512],
                                             start=(k == 0), stop=(k == KTILES - 1))
                    return f

                def rope():
                    ps = ps_box["ps"]
                    if s < SLABS:
                        dst_t = rot.tile([128, 512], F16, tag=f"qtr{s}", name=f"qtr{tb}_{s}")
                        out["QTr"].append(dst_t)
                        dst = dst_t[:]
                    else:
                        dst = KT[:, t_sl]
                    m1 = work.tile([128, 512], F32, tag="m1", name="m1")
                    nc.vector.tensor_mul(m1[:], ps[:], C4[:, t_sl])
                    m2 = work.tile([128, 512], F32, tag="m2", name="m2")
                    for h in (0, 64):
                        nc.vector.tensor_mul(m2[h:h + 32, :], ps[h + 32:h + 64, :],
                                             S4[h:h + 32, t_sl])
                        nc.vector.tensor_mul(m2[h + 32:h + 64, :], ps[h:h + 32, :],
                                             S4[h + 32:h + 64, t_sl])
                    nc.vector.tensor_add(dst, m1[:], m2[:])

                return [mk_mm(0), mk_mm(4), mk_mm(8), mk_mm(12), rope]

            def vgroup(i):
                pv_box = {}

                def mk_mm(k0):
                    def f():
                        if k0 == 0:
                            pv_box["pv"] = ps_acc.tile([128, 128], F32, tag="acc",
                                                       name=f"pv{tb}_{i}",
                                                       padded_shape=[128, 512])
                        pv = pv_box["pv"]
                        for k in range(k0, k0 + 4):
                            nc.tensor.matmul(
                                pv[:], xts[:, k * 512 + i * 128: k * 512 + (i + 1) * 128],
                                WV[:, k * 128:(k + 1) * 128],
                                start=(k == 0), stop=(k == KTILES - 1))
                    return f

                def vcopy():
                    pv = pv_box["pv"]
                    sbi = tb * 4 + i
                    nc.vector.tensor_copy(V0[:, sbi * 65: sbi * 65 + 64], pv[:, 0:64])
                    nc.vector.tensor_copy(V1[:, sbi * 65: sbi * 65 + 64], pv[:, 64:128])
                return [mk_mm(0), mk_mm(4), mk_mm(8), mk_mm(12), vcopy]

            chunks = []
            for s in range(SLABS + 1):
                chunks.extend(group(s))
            for i in range(4):
                chunks.extend(vgroup(i))
            return chunks

        def oproj_chunks(tb, OTNr, t_sl):
            """Filler chunks for the output projection of t-block tb."""
            osb_box = {}

            def mk_ob(ob):
                def f():
                    if ob == 0:
                        osb_box["osb"] = osbp.tile([128, 16 * 512], F16, tag="osb",
                                                   name=f"osb{tb}")
                    po = ps_acc.tile([128, 512], F32, tag="acc", name=f"po{tb}_{ob}")
                    for s in range(SLABS):
                        nc.tensor.matmul(po[:], WO[s][:, ob * 128:(ob + 1) * 128],
                                         OTNr[s][:], start=(s == 0), stop=(s == SLABS - 1))
                    nc.vector.tensor_copy(osb_box["osb"][:, ob * 512:(ob + 1) * 512], po[:])
                    if ob == 7 or ob == 15:
                        h0 = ob - 7
                        outv = outT.rearrange("(o p) t -> p o t", p=128)[:, :, t_sl]
                        osbv = osb_box["osb"][:].rearrange("p (o t) -> p o t", o=16)
                        nc.sync.dma_start(outv[:, h0:ob + 1], osbv[:, h0:ob + 1])
                return f
            return [mk_ob(ob) for ob in range(16)]

        def emit_attention(qc, QTr, filler):
            """Attention for q chunk qc; emits filler chunks between blocks."""
            nblk = (qc + 1) * 4
            total_units = SLABS * (nblk + 3)
            fi = [0]
            ui = [0]

            def fill():
                ui[0] += 1
                want = (ui[0] * len(filler)) // total_units
                while fi[0] < want:
                    filler[fi[0]]()
                    fi[0] += 1

            OTNr = []
            for s in range(SLABS):
                ot0 = ps_ot.tile([65, 512], F32, tag="ot", name=f"ot0_{qc}_{s}")
                ot1 = ps_ot.tile([65, 512], F32, tag="ot", name=f"ot1_{qc}_{s}")

                def score_pair(b, o, sc):
                    nc.tensor.matmul(sc[:, o:512],
                                     KT[0:64, b * 128:(b + 1) * 128],
                                     QTr[s][0:64, o:512], start=True, stop=True)
                    nc.tensor.matmul(sc[:, 512 + o:1024],
                                     KT[64:128, b * 128:(b + 1) * 128],
                                     QTr[s][64:128, o:512], start=True, stop=True)

                def pv_pair(b, o, pt_ap0, pt_ap1, first, last):
                    nc.tensor.matmul(ot0[:, o:512], V0[:, b * 65: b * 65 + 65],
                                     pt_ap0, start=first, stop=last)
                    nc.tensor.matmul(ot1[:, o:512], V1[:, b * 65: b * 65 + 65],
                                     pt_ap1, start=first, stop=last)

                # off-diagonal key blocks
                for b in range(qc * 4):
                    fill()
                    sc = ps_sc.tile([128, 1024], F32, tag="sc", name="sc")
                    score_pair(b, 0, sc[:])
                    pt = ptp.tile([128, 1024], F16, tag="pt", name="pt")
                    nc.scalar.activation(pt[:], sc[:], EXP)
                    pv_pair(b, 0, pt[:, 0:512], pt[:, 512:1024], b == 0, False)

                # diagonal key blocks (trimmed) + fused triangle mask
                ptd = ptdp.tile([128, 5120], F16, tag="ptd", name="ptd")
                for j in range(4):
                    fill()
                    b = qc * 4 + j
                    o = j * 128
                    sc = ps_sc.tile([128, 1024], F32, tag="sc", name="sc")
                    score_pair(b, o, sc[:])
                    nc.scalar.activation(ptd[:, j * 1024 + o: (j + 1) * 1024],
                                         sc[:, o:1024], EXP)
                fill()
                mb = MSK[:].unsqueeze(1).broadcast_to([128, 4, 128])
                for h in range(2):
                    vh = ptd[:, h * 512: h * 512 + 4 * 1152]
                    vh = vh.rearrange("p (j r) -> p j r", r=1152)[:, :, 0:128]
                    nc.gpsimd.tensor_mul(vh, vh, mb)
                for j in range(4):
                    b = qc * 4 + j
                    o = j * 128
                    c0 = j * 1024
                    pv_pair(b, o, ptd[:, c0 + o: c0 + 512],
                            ptd[:, c0 + 512 + o: c0 + 1024],
                            qc == 0 and j == 0, j == 3)

                fill()
                # release ot0/ot1 quickly: copy unnormalized O and den to SBUF
                otu = misc.tile([128, 512], F32, tag="otu", name="otu")
                nc.vector.tensor_copy(otu[0:64, :], ot0[0:64, :])
                nc.vector.tensor_copy(otu[64:128, :], ot1[0:64, :])
                dsb0 = misc.tile([1, 512], F32, tag="dsb0", name="dsb0")
                dsb1 = misc.tile([1, 512], F32, tag="dsb1", name="dsb1")
                nc.vector.tensor_copy(dsb0[:], ot0[64:65, :])
                nc.vector.tensor_copy(dsb1[:], ot1[64:65, :])
                # normalize on SBUF
                rcf0 = misc.tile([1, 512], F32, tag="rcf0", name="rcf0")
                rcf1 = misc.tile([1, 512], F32, tag="rcf1", name="rcf1")
                nc.vector.reciprocal_approx_fast(rcf0[:], dsb0[:])
                nc.vector.reciprocal_approx_fast(rcf1[:], dsb1[:])
                bc = misc.tile([128, 512], F32, tag="bc", name="bc")
                nc.sync.dma_start(bc[0:64, :],
                                  rcf0[:].unsqueeze(1).broadcast_to([1, 64, 512]))
                nc.sync.dma_start(bc[64:128, :],
                                  rcf1[:].unsqueeze(1).broadcast_to([1, 64, 512]))
                otn = rot.tile([128, 512], F16, tag=f"otnr{s}", name=f"otnr{qc}_{s}")
                OTNr.append(otn)
                nc.gpsimd.tensor_mul(otn[0:64, :], otu[0:64, :], bc[0:64, :])
                nc.gpsimd.tensor_mul(otn[64:128, :], otu[64:128, :], bc[64:128, :])
                fill()
            while fi[0] < len(filler):
                filler[fi[0]]()
                fi[0] += 1
            return OTNr

        # ---------- main emission ----------
        proj_out = {0: {}}
        chunks0 = proj_chunks(0, xts0, proj_out[0])
        for c in chunks0:
            c()
        OTNr_prev = None
        for tb in range(TBLK):
            t_sl = slice(tb * 512, (tb + 1) * 512)
            filler = []
            if OTNr_prev is not None:
                t_sl_prev = slice((tb - 1) * 512, tb * 512)
                filler.extend(oproj_chunks(tb - 1, OTNr_prev, t_sl_prev))
            if tb + 1 < TBLK:
                xts_n = load_xts(tb + 1, f"xt{tb + 1}", nc.sync)
                proj_out[tb + 1] = {}
                filler.extend(proj_chunks(tb + 1, xts_n, proj_out[tb + 1]))
            QTr = proj_out.pop(tb)["QTr"]
            OTNr_prev = emit_attention(tb, QTr, filler)
        # final output projection (tail)
        for c in oproj_chunks(TBLK - 1, OTNr_prev,
                              slice((TBLK - 1) * 512, TBLK * 512)):
            c()

    nc.compile()
    return nc


def _prep_inputs(x, freqs_cos, freqs_sin, wq, wk, wv, wo):
    """Build the 8 per-core input maps (host-side sharding + layout prep)."""
    x = np.asarray(x, dtype=np.float32)
    freqs_cos = np.asarray(freqs_cos, dtype=np.float32)
    freqs_sin = np.asarray(freqs_sin, dtype=np.float32)
    wq = np.asarray(wq, dtype=np.float32)
    wk = np.asarray(wk, dtype=np.float32)
    wv = np.asarray(wv, dtype=np.float32)
    wo = np.asarray(wo, dtype=np.float32)

    # de-interleave permutation within a head: [2j] then [2j+1]
    deint = np.concatenate([np.arange(0, HD, 2), np.arange(1, HD, 2)])

    cosT = freqs_cos.T  # [32, T]
    sinT = freqs_sin.T
    c4 = np.tile(cosT, (4, 1)).astype(np.float32)
    s4 = np.concatenate([-sinT, sinT, -sinT, sinT], axis=0).astype(np.float32)

    vones = np.zeros((128, 16 * 65), dtype=np.float32)
    vones[:, 64::65] = 1.0
    # intra-block causal triangle: keep iff key-row p <= q-col c
    msk = (np.arange(128)[:, None] <= np.arange(128)[None, :]).astype(np.float32)

    in_maps = []
    for core in range(8):
        b, g = divmod(core, 4)
        qheads = [8 * g + s + 4 * h for s in range(SLABS) for h in range(2)]
        kvheads = [2 * g, 2 * g + 1]

        wq_cols = np.concatenate([qh * HD + deint for qh in qheads])
        wk_cols = np.concatenate([kh * HD + deint for kh in kvheads])
        wv_cols = np.concatenate([np.arange(kh * HD, (kh + 1) * HD) for kh in kvheads])
        wo_rows = np.concatenate([np.arange(qh * HD, (qh + 1) * HD) for qh in qheads])

        in_maps.append({
            "xT": _f16(x[b].T),
            "wq": _f16(wq[:, wq_cols] * (1.0 / np.sqrt(HD))),
            "wk": _f16(wk[:, wk_cols]),
            "wv": _f16(wv[:, wv_cols]),
            "wo": _f16(wo[wo_rows, :]),
            "c4": c4,
            "s4": s4,
            "vones": _f16(vones),
            "msk": _f16(msk),
        })
    return in_maps


def kernel(x, freqs_cos, freqs_sin, wq, wk, wv, wo, _trace=False):
    from concourse.bass_utils import run_bass_kernel_spmd

    if "nc" not in _CACHE:
        _CACHE["nc"] = _build()
    nc = _CACHE["nc"]

    in_maps = _prep_inputs(x, freqs_cos, freqs_sin, wq, wk, wv, wo)
    res = run_bass_kernel_spmd(nc, in_maps, core_ids=list(range(8)), trace=_trace)
    _CACHE["last_result"] = res

    out = np.empty((B, T, DIM), dtype=np.float32)
    for b in range(B):
        acc = res.results[4 * b]["outT"].astype(np.float32)
        for g in range(1, 4):
            acc = acc + res.results[4 * b + g]["outT"].astype(np.float32)
        out[b] = acc.T
    return out


# revision 7
# speedup vs baseline: 1.2461x; 1.0931x over previous
"""GQA attention (B=2, T=2048, DIM=2048, NH=32, NKV=8, HD=64) with RoPE, causal,
on 8 TRN2 NeuronCores.

Sharding: data-parallel over B (2) x tensor-parallel over kv-head groups (4).
Core i handles batch i//4 and kv heads {2g, 2g+1} (g = i%4), i.e. q heads
8g..8g+8. wq/wk/wv column-parallel, wo row-parallel; host sums the 4 partial
outputs per batch.

Everything on-device is feature-major ("transposed"): x^T, Q^T, K^T are
[feature, t] so no on-device transposes are needed anywhere.

v3 structure:
- Score matmuls for the two kv halves of a slab are emitted adjacently with
  operands at base partitions 0 and 64, so they run concurrently in disjoint
  PE row groups (tile_position (0,0)/(64,0)) - one 512-cycle span for both.
- Causal trimming: the diagonal key blocks only compute scores/exp/PV for
  q >= block offset; the intra-block triangle is zeroed by one fused strided
  mask multiply per half per (slab, q-chunk) on GPSIMD.
- RoPE: the half-swap is partition-shifted DVE muls reading the Q/K PSUM
  directly; the final add runs on GPSIMD (SBUF-only operands).
- Attention output is copied out of PSUM (unnormalized, f32) right after the
  last PV so the two PSUM accumulators recycle after ~2 DVE copies instead of
  after the whole normalize chain; normalization then runs on SBUF tiles.
- The projection matmuls for t-block tb+1 and the output projection for tb-1
  are emitted as filler chunks BETWEEN attention blocks of tb, so the PE
  always has independent work while the exp/mask chain of the current block
  completes (the Tile scheduler issues greedily in emission order).
- Startup DMAs are chunked (4 k-tiles each) so the first projection matmuls
  start as soon as the first chunks land.
"""

import numpy as np

B, T, DIM = 2, 2048, 2048
NH, NKV, HD = 32, 8, 64
G = 4            # tensor-parallel groups
QH = NH // G     # 8 local q heads
SLABS = 4
KTILES = DIM // 128
TBLK = T // 512

_CACHE = {}


def _f16(x: np.ndarray) -> np.ndarray:
    return np.ascontiguousarray(x, dtype=np.float32).astype(np.float16)


def _build():
    import concourse.bass as bass
    import concourse.mybir as mybir
    import concourse.tile as tile
    from concourse import bacc

    F32 = mybir.dt.float32
    F16 = mybir.dt.float16
    EXP = mybir.ActivationFunctionType.Exp

    nc = bacc.Bacc("TRN2", target_bir_lowering=False, debug=False, num_devices=8)

    xT = nc.dram_tensor("xT", [DIM, T], F16, kind="ExternalInput").ap()
    wq = nc.dram_tensor("wq", [DIM, QH * HD], F16, kind="ExternalInput").ap()
    wk = nc.dram_tensor("wk", [DIM, 2 * HD], F16, kind="ExternalInput").ap()
    wv = nc.dram_tensor("wv", [DIM, 2 * HD], F16, kind="ExternalInput").ap()
    wo = nc.dram_tensor("wo", [QH * HD, DIM], F16, kind="ExternalInput").ap()
    c4 = nc.dram_tensor("c4", [128, T], F32, kind="ExternalInput").ap()
    s4 = nc.dram_tensor("s4", [128, T], F32, kind="ExternalInput").ap()
    vones = nc.dram_tensor("vones", [128, 16 * 65], F16, kind="ExternalInput").ap()
    msk = nc.dram_tensor("msk", [128, 128], F16, kind="ExternalInput").ap()
    outT = nc.dram_tensor("outT", [DIM, T], F16, kind="ExternalOutput").ap()

    from contextlib import ExitStack

    with tile.TileContext(nc) as tc, ExitStack() as ctx:
        # ---------- persistent tiles ----------
        pers = ctx.enter_context(tc.tile_pool(name="pers", bufs=1))
        KT = pers.tile([128, T], F16, tag="kt", name="kt")
        V0 = pers.tile([128, 16 * 65], F16, tag="v0", name="v0")
        V1 = pers.tile([128, 16 * 65], F16, tag="v1", name="v1")
        MSK = pers.tile([128, 128], F16, tag="msk", name="msk_sb")
        C4 = pers.tile([128, T], F32, tag="c4", name="c4_sb")
        S4 = pers.tile([128, T], F32, tag="s4", name="s4_sb")
        WQ = pers.tile([128, KTILES * 512], F16, tag="wq", name="wq_sb")
        WK = pers.tile([128, KTILES * 128], F16, tag="wk", name="wk_sb")
        WV = pers.tile([128, KTILES * 128], F16, tag="wv", name="wv_sb")
        WO = [pers.tile([128, T], F16, tag=f"wo{s}", name=f"wo{s}") for s in range(SLABS)]

        rot = ctx.enter_context(tc.tile_pool(name="rot", bufs=2))
        work = ctx.enter_context(tc.tile_pool(name="work", bufs=3))
        ptp = ctx.enter_context(tc.tile_pool(name="ptp", bufs=4))
        ptdp = ctx.enter_context(tc.tile_pool(name="ptd", bufs=2))
        misc = ctx.enter_context(tc.tile_pool(name="misc", bufs=2))
        osbp = ctx.enter_context(tc.tile_pool(name="osb", bufs=1))
        xtp = ctx.enter_context(tc.tile_pool(name="xt", bufs=2))
        ps_acc = ctx.enter_context(tc.tile_pool(name="ps_acc", bufs=2, space="PSUM"))
        ps_sc = ctx.enter_context(tc.tile_pool(name="ps_sc", bufs=2, space="PSUM"))
        ps_ot = ctx.enter_context(tc.tile_pool(name="ps_ot", bufs=2, space="PSUM"))

        def load_xts(tb, name, queue, nchunk=1):
            t_sl = slice(tb * 512, (tb + 1) * 512)
            xt_t = xtp.tile([128, KTILES * 512], F16, tag="x", name=name)
            src = xT.rearrange("(k p) t -> p k t", p=128)[:, :, t_sl]
            dst = xt_t[:].rearrange("p (k t) -> p k t", k=KTILES)
            kc = KTILES // nchunk
            for c in range(nchunk):
                queue.dma_start(dst[:, c * kc:(c + 1) * kc], src[:, c * kc:(c + 1) * kc])
            return xt_t

        # ---------- startup DMAs (chunked so first matmuls start early) ----
        xts0 = load_xts(0, "xt0", nc.sync, nchunk=4)
        wqv_src = wq.rearrange("(k p) c -> p k c", p=128)
        wqv_dst = WQ[:].rearrange("p (k c) -> p k c", k=KTILES)
        for c in range(4):
            nc.scalar.dma_start(wqv_dst[:, c * 4:(c + 1) * 4], wqv_src[:, c * 4:(c + 1) * 4])
        nc.gpsimd.dma_start(
            WK[:].rearrange("p (k c) -> p k c", k=KTILES),
            wk.rearrange("(k p) c -> p k c", p=128))
        nc.gpsimd.dma_start(
            WV[:].rearrange("p (k c) -> p k c", k=KTILES),
            wv.rearrange("(k p) c -> p k c", p=128))
        nc.gpsimd.dma_start(C4[:], c4)
        nc.gpsimd.dma_start(S4[:], s4)
        nc.gpsimd.dma_start(V0[:], vones)
        nc.gpsimd.dma_start(V1[:], vones)
        nc.gpsimd.dma_start(MSK[:], msk)
        for s in range(SLABS):
            nc.scalar.dma_start(WO[s][:], wo[s * 128:(s + 1) * 128, :])

        def proj_chunks(tb, xts, out):
            """Generate filler chunks (closures) for t-block tb's projections.
            out: dict to receive {"QTr": [...]} once emitted."""
            t_sl = slice(tb * 512, (tb + 1) * 512)
            out["QTr"] = []

            def group(s):
                ps_box = {}

                def mk_mm(k0):
                    def f():
                        if k0 == 0:
                            ps_box["ps"] = ps_acc.tile([128, 512], F32, tag="acc",
                                                       name=f"pq{tb}_{s}")
                        ps = ps_box["ps"]
                        for k in range(k0, k0 + 4):
                            if s < SLABS:
                                lhs = WQ[:, k * 512 + s * 128: k * 512 + (s + 1) * 128]
                            else:
                                lhs = WK[:, k * 128:(k + 1) * 128]
                            nc.tensor.matmul(ps[:], lhs, xts[:, k * 512:(k + 1) * 512],
                                             start=(k == 0), stop=(k == KTILES - 1))
                    return f

                def rope():
                    ps = ps_box["ps"]
                    if s < SLABS:
                        dst_t = rot.tile([128, 512], F16, tag=f"qtr{s}", name=f"qtr{tb}_{s}")
                        out["QTr"].append(dst_t)
                        dst = dst_t[:]
                    else:
                        dst = KT[:, t_sl]
                    m1 = work.tile([128, 512], F32, tag="m1", name="m1")
                    nc.vector.tensor_mul(m1[:], ps[:], C4[:, t_sl])
                    m2 = work.tile([128, 512], F32, tag="m2", name="m2")
                    for h in (0, 64):
                        nc.vector.tensor_mul(m2[h:h + 32, :], ps[h + 32:h + 64, :],
                                             S4[h:h + 32, t_sl])
                        nc.vector.tensor_mul(m2[h + 32:h + 64, :], ps[h:h + 32, :],
                                             S4[h + 32:h + 64, t_sl])
                    nc.vector.tensor_add(dst, m1[:], m2[:])

                return [mk_mm(0), mk_mm(4), mk_mm(8), mk_mm(12), rope]

            def vgroup(i):
                pv_box = {}

                def mk_mm(k0):
                    def f():
                        if k0 == 0:
                            pv_box["pv"] = ps_acc.tile([128, 128], F32, tag="acc",
                                                       name=f"pv{tb}_{i}",
                                                       padded_shape=[128, 512])
                        pv = pv_box["pv"]
                        for k in range(k0, k0 + 4):
                            nc.tensor.matmul(
                                pv[:], xts[:, k * 512 + i * 128: k * 512 + (i + 1) * 128],
                                WV[:, k * 128:(k + 1) * 128],
                                start=(k == 0), stop=(k == KTILES - 1))
                    return f

                def vcopy():
                    pv = pv_box["pv"]
                    sbi = tb * 4 + i
                    nc.vector.tensor_copy(V0[:, sbi * 65: sbi * 65 + 64], pv[:, 0:64])
                    nc.vector.tensor_copy(V1[:, sbi * 65: sbi * 65 + 64], pv[:, 64:128])
                return [mk_mm(0), mk_mm(4), mk_mm(8), mk_mm(12), vcopy]

            chunks = []
            for s in range(SLABS + 1):
                chunks.extend(group(s))
            for i in range(4):
                chunks.extend(vgroup(i))
            return chunks

        def oproj_chunks(tb, OTNr, t_sl):
            """Filler chunks for the output projection of t-block tb."""
            osb_box = {}

            def mk_ob(ob):
                def f():
                    if ob == 0:
                        osb_box["osb"] = osbp.tile([128, 16 * 512], F16, tag="osb",
                                                   name=f"osb{tb}")
                    po = ps_acc.tile([128, 512], F32, tag="acc", name=f"po{tb}_{ob}")
                    for s in range(SLABS):
                        nc.tensor.matmul(po[:], WO[s][:, ob * 128:(ob + 1) * 128],
                                         OTNr[s][:], start=(s == 0), stop=(s == SLABS - 1))
                    nc.vector.tensor_copy(osb_box["osb"][:, ob * 512:(ob + 1) * 512], po[:])
                    if ob == 7 or ob == 15:
                        h0 = ob - 7
                        outv = outT.rearrange("(o p) t -> p o t", p=128)[:, :, t_sl]
                        osbv = osb_box["osb"][:].rearrange("p (o t) -> p o t", o=16)
                        nc.sync.dma_start(outv[:, h0:ob + 1], osbv[:, h0:ob + 1])
                return f
            return [mk_ob(ob) for ob in range(16)]

        def emit_attention(qc, QTr, filler):
            """Attention for q chunk qc; emits filler chunks between blocks."""
            nblk = (qc + 1) * 4
            total_units = SLABS * (nblk + 3)
            fi = [0]
            ui = [0]

            def fill():
                ui[0] += 1
                want = (ui[0] * len(filler)) // total_units
                while fi[0] < want:
                    filler[fi[0]]()
                    fi[0] += 1

            OTNr = []
            for s in range(SLABS):
                ot0 = ps_ot.tile([65, 512], F32, tag="ot", name=f"ot0_{qc}_{s}")
                ot1 = ps_ot.tile([65, 512], F32, tag="ot", name=f"ot1_{qc}_{s}")

                def score_pair(b, o, sc):
                    nc.tensor.matmul(sc[:, o:512],
                                     KT[0:64, b * 128:(b + 1) * 128],
                                     QTr[s][0:64, o:512], start=True, stop=True)
                    nc.tensor.matmul(sc[:, 512 + o:1024],
                                     KT[64:128, b * 128:(b + 1) * 128],
                                     QTr[s][64:128, o:512], start=True, stop=True)

                def pv_pair(b, o, pt_ap0, pt_ap1, first, last):
                    nc.tensor.matmul(ot0[:, o:512], V0[:, b * 65: b * 65 + 65],
                                     pt_ap0, start=first, stop=last)
                    nc.tensor.matmul(ot1[:, o:512], V1[:, b * 65: b * 65 + 65],
                                     pt_ap1, start=first, stop=last)

                # diagonal key blocks first (trimmed): their exp + triangle
                # mask latency hides under the off-diagonal stream below
                ptd = ptdp.tile([128, 5120], F16, tag="ptd", name="ptd")
                for j in range(4):
                    fill()
                    b = qc * 4 + j
                    o = j * 128
                    sc = ps_sc.tile([128, 1024], F32, tag="sc", name="sc")
                    score_pair(b, o, sc[:])
                    nc.scalar.activation(ptd[:, j * 1024 + o: (j + 1) * 1024],
                                         sc[:, o:1024], EXP)
                mb = MSK[:].unsqueeze(1).broadcast_to([128, 4, 128])
                for h in range(2):
                    vh = ptd[:, h * 512: h * 512 + 4 * 1152]
                    vh = vh.rearrange("p (j r) -> p j r", r=1152)[:, :, 0:128]
                    nc.vector.tensor_mul(vh, vh, mb)

                # off-diagonal key blocks (no mask dependency)
                for b in range(qc * 4):
                    fill()
                    sc = ps_sc.tile([128, 1024], F32, tag="sc", name="sc")
                    score_pair(b, 0, sc[:])
                    pt = ptp.tile([128, 1024], F16, tag="pt", name="pt")
                    nc.scalar.activation(pt[:], sc[:], EXP)
                    pv_pair(b, 0, pt[:, 0:512], pt[:, 512:1024], b == 0, False)

                fill()
                # diagonal PVs last (first-executed PV carries start=True)
                for j in range(4):
                    b = qc * 4 + j
                    o = j * 128
                    c0 = j * 1024
                    pv_pair(b, o, ptd[:, c0 + o: c0 + 512],
                            ptd[:, c0 + 512 + o: c0 + 1024],
                            qc == 0 and j == 0, j == 3)

                fill()
                # release ot0/ot1 quickly: copy unnormalized O and den to SBUF
                otu = misc.tile([128, 512], F32, tag="otu", name="otu")
                nc.vector.tensor_copy(otu[0:64, :], ot0[0:64, :])
                nc.vector.tensor_copy(otu[64:128, :], ot1[0:64, :])
                dsb0 = misc.tile([1, 512], F32, tag="dsb0", name="dsb0")
                dsb1 = misc.tile([1, 512], F32, tag="dsb1", name="dsb1")
                nc.vector.tensor_copy(dsb0[:], ot0[64:65, :])
                nc.vector.tensor_copy(dsb1[:], ot1[64:65, :])
                # normalize on SBUF
                rcf0 = misc.tile([1, 512], F32, tag="rcf0", name="rcf0")
                rcf1 = misc.tile([1, 512], F32, tag="rcf1", name="rcf1")
                nc.vector.reciprocal_approx_fast(rcf0[:], dsb0[:])
                nc.vector.reciprocal_approx_fast(rcf1[:], dsb1[:])
                bc = misc.tile([128, 512], F32, tag="bc", name="bc")
                nc.sync.dma_start(bc[0:64, :],
                                  rcf0[:].unsqueeze(1).broadcast_to([1, 64, 512]))
                nc.sync.dma_start(bc[64:128, :],
                                  rcf1[:].unsqueeze(1).broadcast_to([1, 64, 512]))
                otn = rot.tile([128, 512], F16, tag=f"otnr{s}", name=f"otnr{qc}_{s}")
                OTNr.append(otn)
                nc.gpsimd.tensor_mul(otn[0:64, :], otu[0:64, :], bc[0:64, :])
                nc.gpsimd.tensor_mul(otn[64:128, :], otu[64:128, :], bc[64:128, :])
                fill()
            while fi[0] < len(filler):
                filler[fi[0]]()
                fi[0] += 1
            return OTNr

        # ---------- main emission ----------
        proj_out = {0: {}}
        chunks0 = proj_chunks(0, xts0, proj_out[0])
        for c in chunks0:
            c()
        OTNr_prev = None
        for tb in range(TBLK):
            t_sl = slice(tb * 512, (tb + 1) * 512)
            filler = []
            if OTNr_prev is not None:
                t_sl_prev = slice((tb - 1) * 512, tb * 512)
                filler.extend(oproj_chunks(tb - 1, OTNr_prev, t_sl_prev))
            if tb + 1 < TBLK:
                xts_n = load_xts(tb + 1, f"xt{tb + 1}", nc.sync)
                proj_out[tb + 1] = {}
                filler.extend(proj_chunks(tb + 1, xts_n, proj_out[tb + 1]))
            QTr = proj_out.pop(tb)["QTr"]
            OTNr_prev = emit_attention(tb, QTr, filler)
        # final output projection (tail)
        for c in oproj_chunks(TBLK - 1, OTNr_prev,
                              slice((TBLK - 1) * 512, TBLK * 512)):
            c()

    nc.compile()
    return nc


def _prep_inputs(x, freqs_cos, freqs_sin, wq, wk, wv, wo):
    """Build the 8 per-core input maps (host-side sharding + layout prep)."""
    x = np.asarray(x, dtype=np.float32)
    freqs_cos = np.asarray(freqs_cos, dtype=np.float32)
    freqs_sin = np.asarray(freqs_sin, dtype=np.float32)
    wq = np.asarray(wq, dtype=np.float32)
    wk = np.asarray(wk, dtype=np.float32)
    wv = np.asarray(wv, dtype=np.float32)
    wo = np.asarray(wo, dtype=np.float32)

    # de-interleave permutation within a head: [2j] then [2j+1]
    deint = np.concatenate([np.arange(0, HD, 2), np.arange(1, HD, 2)])

    cosT = freqs_cos.T  # [32, T]
    sinT = freqs_sin.T
    c4 = np.tile(cosT, (4, 1)).astype(np.float32)
    s4 = np.concatenate([-sinT, sinT, -sinT, sinT], axis=0).astype(np.float32)

    vones = np.zeros((128, 16 * 65), dtype=np.float32)
    vones[:, 64::65] = 1.0
    # intra-block causal triangle: keep iff key-row p <= q-col c
    msk = (np.arange(128)[:, None] <= np.arange(128)[None, :]).astype(np.float32)

    in_maps = []
    for core in range(8):
        b, g = divmod(core, 4)
        qheads = [8 * g + s + 4 * h for s in range(SLABS) for h in range(2)]
        kvheads = [2 * g, 2 * g + 1]

        wq_cols = np.concatenate([qh * HD + deint for qh in qheads])
        wk_cols = np.concatenate([kh * HD + deint for kh in kvheads])
        wv_cols = np.concatenate([np.arange(kh * HD, (kh + 1) * HD) for kh in kvheads])
        wo_rows = np.concatenate([np.arange(qh * HD, (qh + 1) * HD) for qh in qheads])

        in_maps.append({
            "xT": _f16(x[b].T),
            "wq": _f16(wq[:, wq_cols] * (1.0 / np.sqrt(HD))),
            "wk": _f16(wk[:, wk_cols]),
            "wv": _f16(wv[:, wv_cols]),
            "wo": _f16(wo[wo_rows, :]),
            "c4": c4,
            "s4": s4,
            "vones": _f16(vones),
            "msk": _f16(msk),
        })
    return in_maps


def kernel(x, freqs_cos, freqs_sin, wq, wk, wv, wo, _trace=False):
    from concourse.bass_utils import run_bass_kernel_spmd

    if "nc" not in _CACHE:
        _CACHE["nc"] = _build()
    nc = _CACHE["nc"]

    in_maps = _prep_inputs(x, freqs_cos, freqs_sin, wq, wk, wv, wo)
    res = run_bass_kernel_spmd(nc, in_maps, core_ids=list(range(8)), trace=_trace)
    _CACHE["last_result"] = res

    out = np.empty((B, T, DIM), dtype=np.float32)
    for b in range(B):
        acc = res.results[4 * b]["outT"].astype(np.float32)
        for g in range(1, 4):
            acc = acc + res.results[4 * b + g]["outT"].astype(np.float32)
        out[b] = acc.T
    return out
